# revision 2
# baseline (speedup 1.0000x reference)
"""CodeGen attention on 8 Trainium2 NeuronCores (Bass/Tile) — v4.

Sharding: tensor-parallel over the 4 CodeGen mp head-groups x data-parallel
over batch 2. Core c = dp*4 + tp handles batch dp, head group tp (4 heads).

Structure (all matmul operands bf16, fp32 PSUM accumulation):
  - Q^T/K^T/V resident in SBUF between projection and attention.
  - emission interleaves the two 1024-wide projection slices with the
    attention query chunks they unlock:
       p1(ss0) -> attn(q0) -> attn(q1) -> p1(ss1) -> attn(q2) -> attn(q3)
    so the ss1 X/weight DMAs hide under attention and the first two
    AllGathers complete long before the out-projection needs them.
  - per-512-query-chunk bf16 AllGather within each batch group of 4.
  - attention software-pipelines two heads (head b one k-tile behind) so
    each exp has ~1us of other-head PE work as lead time.
  - W_out + first out-projection tiles prefetch during late attention;
    emission order keeps semaphore-gated DMAs from head-of-line blocking
    the sync queue.

Host assembles the [B, S, D] output from per-core [S, D/4] column shards.
"""

import numpy as np

B, S, D = 2, 2048, 4096
N_HEAD = 16
HD = 256
MP = 4
ROT = 64
LOCAL = D // MP            # 1024 (= 4 heads * 256)
DT = D // 128              # 32 contraction tiles
N_CORES = 8
H_LOC = N_HEAD // MP       # 4 heads per core
NQ = 4                     # 512-query chunks

_CACHE = {}


def _emit_body(nc, tc, tens, psp, with_collective, rep):
    """One full pipeline pass. rep only namespaces DRAM scratch."""
    import concourse.tile as tile  # noqa: F401
    from concourse import mybir

    f32 = mybir.dt.float32
    f32r = mybir.dt.float32r
    bf16 = mybir.dt.bfloat16
    EXP = mybir.ActivationFunctionType.Exp

    (xt_r, wq_d, wk_d, wv_r, wo_r, cost, sint, masks, y,
     og_in, og_out, rt_sb, ones_sb, onesr_sb) = tens

    with tc.tile_pool(name="qkv", bufs=1) as qkvp, \
         tc.tile_pool(name="etp", bufs=4) as etp, \
         tc.tile_pool(name="ogp", bufs=3) as ogp, \
         tc.tile_pool(name="rbp", bufs=2) as rbp, \
         tc.tile_pool(name="rip", bufs=2) as rip, \
         tc.tile_pool(name="mkp", bufs=1) as mkp:
        qT_sb = qkvp.tile([128, 2 * H_LOC, S], bf16, name="qT_sb")
        kT_sb = qkvp.tile([128, 2 * H_LOC, S], bf16, name="kT_sb")
        v_sb = qkvp.tile([128, S // 128, LOCAL], bf16, name="v_sb")
        masks_sb = mkp.tile([128, 4, 512], bf16, name="masks_sb")
        nc.sync.dma_start(masks_sb[:], masks.ap())

        # ---------------- phase 2 helper: one query chunk ----------------
        def attn_chunk(qn):
            nk = (qn + 1) * 4
            q0 = qn * 512
            for h2 in range(H_LOC // 2):
                # two heads software-pipelined: head b one k-tile behind
                ha, hb = 2 * h2, 2 * h2 + 1
                hs = (ha, hb)
                rs_pair = psp.tile([64, 512], f32, name="rs_pair",
                                   tag="ps")
                rs = {ha: rs_pair[0:1, :], hb: rs_pair[32:33, :]}
                ov = {h: [psp.tile([128, 512], f32, name=f"ov{h}_{dm}",
                                   tag="ps")
                          for dm in range(2)]
                      for h in hs}

                def _c0(ki):
                    # columns < (ki-qn*4)*128 of a diagonal k-tile are
                    # fully masked: skip them in every matmul/activation
                    return max(0, (ki - qn * 4) * 128)

                def scores_exp(h, ki):
                    c0 = _c0(ki)
                    sp = psp.tile([128, 512], f32, name="sp", tag="ps")
                    for dd in range(2):
                        nc.tensor.matmul(
                            sp[:, c0:512],
                            kT_sb[:, h * 2 + dd, ki * 128:(ki + 1) * 128],
                            qT_sb[:, h * 2 + dd, q0 + c0:q0 + 512],
                            start=(dd == 0), stop=(dd == 1))
                    et = etp.tile([128, 512], bf16, name="et", tag="et")
                    nc.scalar.activation(et[:, c0:512], sp[:, c0:512], EXP,
                                         bias=0.0, scale=1.0 / 16.0)
                    if ki >= qn * 4:
                        nc.vector.tensor_mul(
                            et[:, c0:512], et[:, c0:512],
                            masks_sb[:, ki - qn * 4, c0:512])
                    return et

                def pv(h, ki, et):
                    c0 = _c0(ki)
                    nc.tensor.matmul(rs[h][:, c0:512], ones_sb[:],
                                     et[:, c0:512],
                                     start=(ki == 0), stop=(ki == nk - 1))
                    for dm in range(2):
                        nc.tensor.matmul(
                            ov[h][dm][:, c0:512],
                            v_sb[:, ki,
                                 h * HD + dm * 128:h * HD + (dm + 1) * 128],
                            et[:, c0:512], start=(ki == 0),
                            stop=(ki == nk - 1))

                et_b = None
                for ki in range(nk):
                    et_a = scores_exp(ha, ki)
                    if et_b is not None:
                        pv(hb, ki - 1, et_b)
                    et_b = scores_exp(hb, ki)
                    pv(ha, ki, et_a)
                pv(hb, nk - 1, et_b)

                # normalize tail, ping-pong-free: both reciprocals
                # first (DVE), then both broadcast matmuls (PE), then the
                # copies and scaling muls — so neither engine waits on the
                # other's second iteration
                rinv = {}
                for h in hs:
                    rinv[h] = rip.tile([1, 512], f32, name="rinv",
                                       tag="rinv")
                    nc.vector.reciprocal(rinv[h][:], rs[h][:])
                rb_sb = {}
                for h in hs:
                    # partition-broadcast on the (otherwise idle) GPSIMD
                    # engine: keeps the whole normalize tail off the PE
                    rb_sb[h] = rbp.tile([128, 512], f32, name="rb_sb",
                                        tag="rb")
                    nc.gpsimd.partition_broadcast(rb_sb[h][:], rinv[h][:])
                for h in hs:
                    for dm in range(2):
                        og_sb = ogp.tile([128, 512], bf16, name="og_sb",
                                         tag="og")
                        nc.vector.tensor_mul(og_sb[:], ov[h][dm][:],
                                             rb_sb[h][:])
                        rr = (h * 2 + dm) * 128
                        # scalar-engine DGE ring: keeps the sync queue
                        # load-only (no head-of-line data waits)
                        nc.scalar.dma_start(og_in[qn, rr:rr + 128, :],
                                            og_sb[:])
            if with_collective:
                nc.gpsimd.collective_compute(
                    "AllGather",
                    mybir.AluOpType.bypass,
                    replica_groups=[[0, 1, 2, 3], [4, 5, 6, 7]],
                    ins=[og_in[qn].opt()],
                    outs=[og_out[qn].opt()],
                )
            else:
                for blk in range(MP):
                    nc.sync.dma_start(
                        og_out[qn, blk * LOCAL:(blk + 1) * LOCAL, :],
                        og_in[qn])

        # ---------------- phase 1 (one 1024-wide s slice) ----------------
        with tc.tile_pool(name="xrp", bufs=1) as xrp, \
             tc.tile_pool(name="wmp", bufs=2) as wmp, \
             tc.tile_pool(name="sqp", bufs=2) as sqp, \
             tc.tile_pool(name="t1p", bufs=1) as t1p, \
             tc.tile_pool(name="t2p", bufs=1) as t2p, \
             tc.tile_pool(name="wvp", bufs=2) as wvp, \
             tc.tile_pool(name="trig", bufs=1) as trig:
            # trig tables folded to 128 partitions: rows 0:64 = cols 0:1024,
            # rows 64:128 = cols 1024:2048
            cost_sb = trig.tile([128, 1024], bf16, name="cost_sb")
            nc.sync.dma_start(cost_sb[:], cost.ap())
            sint_sb = trig.tile([128, 1024], bf16, name="sint_sb")
            nc.sync.dma_start(sint_sb[:], sint.ap())

            def trig_slice(t, sg):
                r0 = 0 if sg < 1024 else ROT
                c0 = sg % 1024
                return t[r0:r0 + ROT, c0:c0 + 512]

            def load_xr(ss):
                xr = xrp.tile([128, DT, 1024], bf16, name="xr", tag="xr")
                for db in range(8):
                    nc.sync.dma_start(
                        xr[:, db * 4:(db + 1) * 4, :],
                        xt_r[:, db * 4:(db + 1) * 4,
                             ss * 1024:(ss + 1) * 1024])
                return xr

            def load_wm(m):
                w_d = wq_d if m < 2 * H_LOC else wk_d
                wm = wmp.tile([128, DT, 128], bf16, name="wm", tag="wm")
                nc.sync.dma_start(wm[:], w_d[m % (2 * H_LOC)])
                return wm

            def p1_slice(ss, xr, wms):
                # Q^T / K^T (W stationary, X^T moving), rotary fused.
                # dt-outer / n-inner: both 512-wide n slices reuse the same
                # stationary weight tile -> half the LDWEIGHTS traffic.
                for m in range(2 * 2 * H_LOC):
                    mm = m % (2 * H_LOC)
                    wm = wms[m] if wms is not None else load_wm(m)
                    dest = qT_sb if m < 2 * H_LOC else kT_sb
                    ps2 = [psp.tile([128, 512], f32, name=f"ps{n}",
                                    tag="ps")
                           for n in range(2)]
                    for dt in range(DT):
                        for n in range(2):
                            nc.tensor.matmul(
                                ps2[n][:], wm[:, dt, :],
                                xr[:, dt, n * 512:(n + 1) * 512],
                                start=(dt == 0), stop=(dt == DT - 1))
                    for n in range(2):
                        ps = ps2[n]
                        sg = ss * 1024 + n * 512
                        if mm % 2 == 0:
                            # rows 0:64 are the rotary dims of a head
                            sq = sqp.tile([ROT, 512], bf16, name="sq",
                                          tag="sq")
                            nc.vector.tensor_copy(sq[:], ps[0:ROT, :])
                            rp = psp.tile([128, 512], f32, name="rp",
                                          tag="ps")
                            nc.tensor.matmul(rp[0:ROT, :], rt_sb[:], sq[:],
                                             start=True, stop=True)
                            t1 = t1p.tile([ROT, 512], f32, name="t1",
                                          tag="t1")
                            nc.vector.tensor_mul(t1[:], ps[0:ROT, :],
                                                 trig_slice(cost_sb, sg))
                            t2 = t2p.tile([ROT, 512], f32, name="t2",
                                          tag="t2")
                            nc.vector.tensor_mul(t2[:], rp[0:ROT, :],
                                                 trig_slice(sint_sb, sg))
                            nc.vector.tensor_add(
                                dest[0:ROT, mm, sg:sg + 512], t1[:], t2[:])
                            nc.vector.tensor_copy(
                                dest[ROT:128, mm, sg:sg + 512],
                                ps[ROT:128, :])
                        else:
                            nc.vector.tensor_copy(
                                dest[:, mm, sg:sg + 512], ps[:])

                # V natural layout (X^T stationary, Wv moving)
                for vn in range(2):
                    pss = [psp.tile([128, 512], f32, name=f"vps{sm}",
                                    tag="ps")
                           for sm in range(8)]
                    for dt in range(DT):
                        wvt = wvp.tile([128, 512], bf16, name="wvt",
                                       tag="wvt")
                        nc.sync.dma_start(
                            wvt[:], wv_r[:, dt, vn * 512:(vn + 1) * 512])
                        for sm in range(8):
                            nc.tensor.matmul(
                                pss[sm][:],
                                xr[:, dt, sm * 128:(sm + 1) * 128],
                                wvt[:], start=(dt == 0),
                                stop=(dt == DT - 1))
                    for sm in range(8):
                        nc.vector.tensor_copy(
                            v_sb[:, ss * 8 + sm, vn * 512:(vn + 1) * 512],
                            pss[sm][:])

            # ---- epoch A: slice 0, then attention chunks 0/1 while the
            # slice-1 inputs stream in ----
            wm0a = load_wm(0)
            xr0 = load_xr(0)
            wms0 = [wm0a] + [load_wm(m) for m in range(1, 2 * 2 * H_LOC)]
            p1_slice(0, xr0, wms0)
            xr1 = load_xr(1)          # waits WAR on xr0; hides under attn
            # emit every ss1 weight load now: the sync queue is load-only,
            # so these stream in as pool slots free during attention
            wms1 = [load_wm(m) for m in range(2 * 2 * H_LOC)]
            attn_chunk(0)
            attn_chunk(1)
            p1_slice(1, xr1, wms1)

        # ---- epoch B: attention chunks 2/3 + out-projection ----
        # wo/om pools reuse the phase-1 space; the W_out load and the first
        # out-projection tiles prefetch during attention.
        with tc.tile_pool(name="wop", bufs=1) as wop, \
             tc.tile_pool(name="omp", bufs=3) as omp, \
             tc.tile_pool(name="resp", bufs=2) as resp:
            wo_sb = wop.tile([128, DT, LOCAL], bf16, name="wo_sb")
            for db in range(4):
                nc.sync.dma_start(wo_sb[:, db * 8:(db + 1) * 8, :],
                                  wo_r[:, db * 8:(db + 1) * 8, :])
            # prefetch the first two out-projection tiles (AllGather 0
            # completed during phase-1 slice 1)
            og_r0 = og_out[0].rearrange("(dt p) s -> p dt s", p=128)
            om_pre = []
            for st in range(3):
                om = omp.tile([128, DT, 128], bf16, name="om", tag="om")
                nc.sync.dma_start(om[:],
                                  og_r0[:, :, st * 128:(st + 1) * 128])
                om_pre.append(om)

            attn_chunk(2)
            attn_chunk(3)

            # ---------------- phase 3: out projection ----------------
            for c in range(NQ):
                og_r = og_out[c].rearrange("(dt p) s -> p dt s", p=128)
                for st in range(4):
                    if c == 0 and st < 3:
                        om = om_pre[st]
                    else:
                        om = omp.tile([128, DT, 128], bf16, name="om",
                                      tag="om")
                        nc.sync.dma_start(
                            om[:], og_r[:, :, st * 128:(st + 1) * 128])
                    for cn in range(2):
                        ps3 = psp.tile([128, 512], f32, name="ps3",
                                       tag="ps")
                        for dt in range(DT):
                            nc.tensor.matmul(
                                ps3[:], om[:, dt, :],
                                wo_sb[:, dt, cn * 512:(cn + 1) * 512],
                                start=(dt == 0), stop=(dt == DT - 1))
                        res = resp.tile([128, 512], f32, name="res",
                                        tag="res")
                        nc.vector.tensor_copy(res[:], ps3[:])
                        nc.scalar.dma_start(
                            y.ap()[c * 512 + st * 128:
                                   c * 512 + (st + 1) * 128,
                                   cn * 512:(cn + 1) * 512], res[:])


def _build_program(with_collective=True, n_repeat=1):
    import concourse.bass as bass  # noqa: F401
    import concourse.tile as tile
    from concourse import bacc, mybir

    f32 = mybir.dt.float32
    f32r = mybir.dt.float32r
    bf16 = mybir.dt.bfloat16

    nc = bacc.Bacc("TRN2", target_bir_lowering=False, debug=False,
                   enable_asserts=True, num_devices=N_CORES)

    xt = nc.dram_tensor("xt", [D, S], bf16, kind="ExternalInput")
    wq = nc.dram_tensor("wq", [2 * H_LOC, 128, DT, 128], bf16,
                        kind="ExternalInput")
    wk = nc.dram_tensor("wk", [2 * H_LOC, 128, DT, 128], bf16,
                        kind="ExternalInput")
    wv = nc.dram_tensor("wv", [D, LOCAL], bf16, kind="ExternalInput")
    wo = nc.dram_tensor("wo", [D, LOCAL], bf16, kind="ExternalInput")
    cost = nc.dram_tensor("cost", [128, 1024], bf16, kind="ExternalInput")
    sint = nc.dram_tensor("sint", [128, 1024], bf16, kind="ExternalInput")
    rt = nc.dram_tensor("rt", [ROT, ROT], bf16, kind="ExternalInput")
    ones = nc.dram_tensor("ones", [128, 1], bf16, kind="ExternalInput")
    onesr = nc.dram_tensor("onesr", [1, 128], f32r, kind="ExternalInput")
    masks = nc.dram_tensor("masks", [128, 4, 512], bf16,
                           kind="ExternalInput")
    y = nc.dram_tensor("y", [S, LOCAL], f32, kind="ExternalOutput")

    xt_r = xt.ap().rearrange("(dt p) s -> p dt s", p=128)
    wq_d = wq.ap()
    wk_d = wk.ap()
    wv_r = wv.ap().rearrange("(dt p) c -> p dt c", p=128)
    wo_r = wo.ap().rearrange("(dt p) c -> p dt c", p=128)

    with tile.TileContext(nc) as tc:
        with tc.tile_pool(name="dram", bufs=1, space="DRAM") as dpool, \
             tc.tile_pool(name="const", bufs=1) as cpool, \
             tc.tile_pool(name="psum", bufs=8, space="PSUM") as psp:
            og_in = dpool.tile([NQ, LOCAL, 512], bf16, name="og_in")
            og_out = dpool.tile([NQ, MP * LOCAL, 512], bf16, name="og_out")

            rt_sb = cpool.tile([ROT, ROT], bf16, name="rt_sb")
            nc.sync.dma_start(rt_sb[:], rt.ap())
            ones_sb = cpool.tile([128, 1], bf16, name="ones_sb")
            nc.sync.dma_start(ones_sb[:], ones.ap())
            onesr_sb = cpool.tile([1, 128], f32r, name="onesr_sb")
            nc.sync.dma_start(onesr_sb[:], onesr.ap())

            tens = (xt_r, wq_d, wk_d, wv_r, wo_r, cost, sint, masks, y,
                    og_in, og_out, rt_sb, ones_sb, onesr_sb)
            for rep in range(n_repeat):
                _emit_body(nc, tc, tens, psp, with_collective, rep)

    nc.compile()
    return nc


def _rotary_tables(position_ids):
    """sin/cos tables folded to [128, 1024] bf16 per batch: rows 0:64 =
    cols 0:1024 of the [64, S] transposed table, rows 64:128 = cols
    1024:2048."""
    import ml_dtypes
    pos = np.asarray(position_ids).astype(np.int64)
    inv_freq = 1.0 / (10000.0 ** (np.arange(0, ROT, 2, dtype=np.float32) / ROT))
    sinusoid = np.arange(2048, dtype=np.float32)[:, None] * inv_freq[None, :]
    sin_t = np.sin(sinusoid).astype(np.float32)   # [2048, 32]
    cos_t = np.cos(sinusoid).astype(np.float32)

    def fold(t):
        return np.concatenate([t[:, 0:1024], t[:, 1024:2048]], axis=0)

    outs = []
    for b in range(pos.shape[0]):
        sg = np.repeat(sin_t[pos[b]], 2, axis=1).T   # [64, S]
        cg = np.repeat(cos_t[pos[b]], 2, axis=1).T
        outs.append(
            (np.ascontiguousarray(fold(sg)).astype(ml_dtypes.bfloat16),
             np.ascontiguousarray(fold(cg)).astype(ml_dtypes.bfloat16)))
    return outs


def _consts():
    import ml_dtypes
    bf16 = ml_dtypes.bfloat16
    rt_np = np.zeros((ROT, ROT), dtype=np.float32)
    for i in range(ROT // 2):
        rt_np[2 * i + 1, 2 * i] = -1.0   # rt = R^T for rotate_every_two
        rt_np[2 * i, 2 * i + 1] = 1.0
    ones_np = np.ones((128, 1), dtype=bf16)
    onesr_np = np.ones((1, 128), dtype=np.float32)
    masks_np = np.zeros((128, 4, 512), dtype=np.float32)
    ii = np.arange(128)[:, None]
    qq = np.arange(512)[None, :]
    for j in range(4):
        masks_np[:, j, :] = (128 * j + ii <= qq).astype(np.float32)
    return (rt_np.astype(bf16), onesr_np, ones_np, masks_np.astype(bf16))


def _wqk_tiles(w):
    """[D, LOCAL] f32 -> [8, 128, DT, 128] bf16 pre-tiled for 1MB DMAs."""
    import ml_dtypes
    t = w.reshape(DT, 128, 2 * H_LOC, 128).transpose(2, 1, 0, 3)
    return np.ascontiguousarray(t).astype(ml_dtypes.bfloat16)


def _in_maps(hidden_states, position_ids, W_qkv, W_out):
    import ml_dtypes
    bf16 = ml_dtypes.bfloat16
    hs = np.asarray(hidden_states, dtype=np.float32)
    wqkv = np.asarray(W_qkv, dtype=np.float32)
    wout = np.asarray(W_out, dtype=np.float32)
    rt_np, onesr_np, ones_np, masks_np = _consts()
    trig = _rotary_tables(position_ids)

    xts = [np.ascontiguousarray(hs[b].T).astype(bf16) for b in range(B)]
    in_maps = []
    for c in range(N_CORES):
        dp, tp = c // MP, c % MP
        wl = wqkv[:, tp * 3 * LOCAL:(tp + 1) * 3 * LOCAL]
        sg, cg = trig[dp]
        in_maps.append({
            "xt": xts[dp],
            "wq": _wqk_tiles(wl[:, 0:LOCAL]),
            "wv": np.ascontiguousarray(wl[:, LOCAL:2 * LOCAL]).astype(bf16),
            "wk": _wqk_tiles(wl[:, 2 * LOCAL:3 * LOCAL]),
            "wo": np.ascontiguousarray(
                wout[:, tp * LOCAL:(tp + 1) * LOCAL]).astype(bf16),
            "cost": cg, "sint": sg,
            "rt": rt_np, "ones": ones_np, "onesr": onesr_np,
            "masks": masks_np,
        })
    return in_maps


def _get_runner(n_repeat=1):
    key = ("runner", n_repeat)
    if key in _CACHE:
        return _CACHE[key]
    import jax
    from jax.sharding import Mesh, PartitionSpec, NamedSharding
    from jax.experimental.shard_map import shard_map
    from concourse import bass2jax, mybir

    nc = _build_program(with_collective=True, n_repeat=n_repeat)
    bass2jax.install_neuronx_cc_hook()

    partition_name = (nc.partition_id_tensor.name
                      if nc.partition_id_tensor else None)
    in_names, out_names, out_avals, zero_outs = [], [], [], []
    for alloc in nc.m.functions[0].allocations:
        if not isinstance(alloc, mybir.MemoryLocationSet):
            continue
        name = alloc.memorylocations[0].name
        if alloc.kind == "ExternalInput":
            if name != partition_name:
                in_names.append(name)
        elif alloc.kind == "ExternalOutput":
            shape = tuple(alloc.tensor_shape)
            dtype = mybir.dt.np(alloc.dtype)
            out_names.append(name)
            out_avals.append(jax.core.ShapedArray(shape, dtype))
            zero_outs.append(np.zeros(shape, dtype))
    n_params = len(in_names)
    all_names = in_names + out_names
    if partition_name is not None:
        all_names = all_names + [partition_name]

    def _body(*args):
        operands = list(args)
        if partition_name is not None:
            operands.append(bass2jax.partition_id_tensor())
        outs = bass2jax._bass_exec_p.bind(
            *operands,
            out_avals=tuple(out_avals),
            in_names=tuple(all_names),
            out_names=tuple(out_names),
            lowering_input_output_aliases=(),
            sim_require_finite=True,
            sim_require_nnan=True,
            nc=nc,
        )
        return tuple(outs)

    devices = jax.devices()[:N_CORES]
    mesh = Mesh(np.asarray(devices), ("core",))
    n_outs = len(out_names)
    sharded = jax.jit(
        shard_map(_body, mesh=mesh,
                  in_specs=(PartitionSpec("core"),) * (n_params + n_outs),
                  out_specs=(PartitionSpec("core"),) * n_outs,
                  check_rep=False),
        keep_unused=True,
    )
    sharding = NamedSharding(mesh, PartitionSpec("core"))
    runner = {
        "nc": nc, "sharded": sharded, "in_names": in_names,
        "out_names": out_names, "out_avals": out_avals,
        "zero_outs": zero_outs, "sharding": sharding, "jax": jax,
    }
    _CACHE[key] = runner
    return runner


def _stage(runner, in_maps):
    jax = runner["jax"]
    concat_in = [
        np.concatenate([np.asarray(in_maps[c][name]) for c in range(N_CORES)],
                       axis=0)
        for name in runner["in_names"]
    ]
    concat_zero = [
        np.zeros((N_CORES * z.shape[0], *z.shape[1:]), z.dtype)
        for z in runner["zero_outs"]
    ]
    return [jax.device_put(a, runner["sharding"]) for a in concat_in + concat_zero]


def _execute(runner, staged):
    jax = runner["jax"]
    outs = runner["sharded"](*staged)
    outs = jax.block_until_ready(outs)
    return outs


def kernel(hidden_states, position_ids, W_qkv, W_out):
    runner = _get_runner()
    in_maps = _in_maps(hidden_states, position_ids, W_qkv, W_out)
    staged = _stage(runner, in_maps)
    outs = _execute(runner, staged)
    yc = np.asarray(outs[0]).reshape(N_CORES, S, LOCAL)
    result = np.empty((B, S, D), dtype=np.float32)
    for c in range(N_CORES):
        dp, tp = c // MP, c % MP
        result[dp][:, tp * LOCAL:(tp + 1) * LOCAL] = yc[c]
    return result


def bench(inputs, iters=10, n_repeat=1):
    """Return per-call wall-clock seconds (list) for the staged executable."""
    import time
    runner = _get_runner(n_repeat)
    in_maps = _in_maps(**inputs)
    staged = _stage(runner, in_maps)
    _execute(runner, staged)  # warm-up / compile
    times = []
    for _ in range(iters):
        t0 = time.perf_counter()
        _execute(runner, staged)
        times.append(time.perf_counter() - t0)
    return times
